# revision 4
# baseline (speedup 1.0000x reference)
"""CrossModalAttention Trainium2 kernel (8 NeuronCores, SPMD).

Problem (hardcoded shapes): B=2, C=256, H=W=64 (N=4096), 8 heads x d=32,
1x1-conv QKV projections from query/key feature maps, softmax attention,
output projection, residual + GroupNorm(32 groups).

Sharding: core c handles batch b=c//4 and query-token slice
[1024*(c%4), 1024*(c%4+1)).  K/V are computed per-core for the full token
range (cheap), attention + out-projection + residual are fully local, and
GroupNorm spatial statistics are combined with a tiny (2KB) AllReduce
across the 4 cores of each batch group.

Device algorithm highlights:
  - S^T-layout flash attention: scores are built transposed [m, nq] so the
    PV contraction needs no transposes anywhere.
  - No-max softmax (scores are ~N(0,1); exp never overflows in f32),
    exp() runs on the ACT engine straight out of PSUM, one [128, 2048]
    call per 128-token m-chunk (4 heads at once, row-packed QK matmuls
    via tile_position).
  - The softmax denominator Z is computed by a ones[128,32] matmul that
    also REPLICATES Z across the 32 partitions of each head strip, so the
    final normalization is a plain aligned tensor_mul.
"""

import math

import numpy as np

import concourse.bass as bass
import concourse.mybir as mybir
import concourse.tile as tile
from concourse.bass_utils import run_bass_kernel_spmd

P = 128
C = 256
N = 4096
NQ = 1024  # per-core query-token slice
NB = 512  # nq block (PSUM free-dim)
H = 8
D = 32
MCH = N // P  # 32 m-chunks
SCALE = 1.0 / math.sqrt(D)
EPS = 1e-5
F32 = mybir.dt.float32

_REPLICA_GROUPS = [[0, 1, 2, 3], [4, 5, 6, 7]]


def _legalize_waits(nc):
    """The walrus build in this container accepts at most 1 sync wait per
    instruction (2 for EventSemaphore), but Tile's wait-assignment attaches
    more.  Split the excess onto EventSemaphore carrier instructions
    inserted immediately before the violating instruction on the same
    engine (engines execute in order, so this is equivalent).
    """
    import bass_rust

    n = 0
    for f in nc.m.functions:
        for bb in f.blocks:
            insts = list(bb.instructions)
            out = []
            changed = False
            for ins in insts:
                cap = 2 if ins.opcode == "EventSemaphore" else 1
                si = ins.sync_info
                ow = list(si.on_wait) if si and si.on_wait else []
                if len(ow) > cap:
                    keep, excess = ow[:cap], ow[cap:]
                    for ci in range(0, len(excess), 2):
                        ev = bass_rust.InstEventSemaphore(
                            name=f"wfix_{ins.name}_{n}_{ci}", ins=[], outs=[]
                        )
                        ev.engine = ins.engine
                        ev.sync_info = mybir.SyncInfo(
                            on_wait=list(excess[ci : ci + 2]), on_update=[]
                        )
                        out.append(ev)
                    si.on_wait = keep
                    changed = True
                    n += 1
                out.append(ins)
            if changed:
                bb.instructions = out
    return n


def build_kernel():
    nc = bass.Bass(num_devices=8)

    # ---- kernel I/O (per-core) ----
    qf = nc.dram_tensor("qf", [C, NQ], F32, kind="ExternalInput")
    kf = nc.dram_tensor("kf", [C, N], F32, kind="ExternalInput")
    qwT = nc.dram_tensor("qwT", [C, C], F32, kind="ExternalInput")
    kwT = nc.dram_tensor("kwT", [C, C], F32, kind="ExternalInput")
    vwT = nc.dram_tensor("vwT", [C, C], F32, kind="ExternalInput")
    owT = nc.dram_tensor("owT", [C, C], F32, kind="ExternalInput")
    qb = nc.dram_tensor("qb", [P, 2], F32, kind="ExternalInput")
    kb = nc.dram_tensor("kb", [P, 2], F32, kind="ExternalInput")
    ob = nc.dram_tensor("ob", [P, 2], F32, kind="ExternalInput")
    vb = nc.dram_tensor("vb", [1, C], F32, kind="ExternalInput")
    gnw = nc.dram_tensor("gnw", [P, 2], F32, kind="ExternalInput")
    gnb = nc.dram_tensor("gnb", [P, 2], F32, kind="ExternalInput")
    y = nc.dram_tensor("y", [C, NQ], F32, kind="ExternalOutput")

    cc_in = nc.dram_tensor("cc_in", [P, 4], F32)
    cc_out = nc.dram_tensor("cc_out", [P, 4], F32)

    # block-diagonal group-sum matrices (8 channels per group)
    bd_np = np.zeros((P, 16), np.float32)
    for g in range(16):
        bd_np[8 * g : 8 * g + 8, g] = 1.0
    bd_dram = nc.inline_tensor(bd_np, name="bd")
    bdT_dram = nc.inline_tensor(np.ascontiguousarray(bd_np.T), name="bdT")

    with tile.TileContext(nc) as tc:
        with (
            tc.tile_pool(name="consts", bufs=1) as consts,
            tc.tile_pool(name="big", bufs=1) as big,
            tc.tile_pool(name="work", bufs=2) as work,
            tc.tile_pool(name="expp", bufs=3) as expp,
            tc.tile_pool(name="psum", bufs=1, space="PSUM") as psum,
        ):
            # ---- load inputs ----
            qf_sb = big.tile([P, 2, NQ], F32)
            nc.sync.dma_start(out=qf_sb, in_=qf[:].rearrange("(c p) n -> p c n", p=P))
            kf_sb = big.tile([P, 2, N], F32)
            nc.sync.dma_start(out=kf_sb, in_=kf[:].rearrange("(c p) n -> p c n", p=P))

            wq = consts.tile([P, 2, C], F32)
            nc.sync.dma_start(out=wq, in_=qwT[:].rearrange("(c p) o -> p c o", p=P))
            wk = consts.tile([P, 2, C], F32)
            nc.sync.dma_start(out=wk, in_=kwT[:].rearrange("(c p) o -> p c o", p=P))
            wv = consts.tile([P, 2, C], F32)
            nc.sync.dma_start(out=wv, in_=vwT[:].rearrange("(c p) o -> p c o", p=P))
            wo = consts.tile([P, 2, C], F32)
            nc.sync.dma_start(out=wo, in_=owT[:].rearrange("(c p) o -> p c o", p=P))

            qb_sb = consts.tile([P, 2], F32)
            nc.sync.dma_start(out=qb_sb, in_=qb[:])
            kb_sb = consts.tile([P, 2], F32)
            nc.sync.dma_start(out=kb_sb, in_=kb[:])
            ob_sb = consts.tile([P, 2], F32)
            nc.sync.dma_start(out=ob_sb, in_=ob[:])
            gnw_sb = consts.tile([P, 2], F32)
            nc.sync.dma_start(out=gnw_sb, in_=gnw[:])
            gnb_sb = consts.tile([P, 2], F32)
            nc.sync.dma_start(out=gnb_sb, in_=gnb[:])
            vb_sb = consts.tile([1, C], F32)
            nc.sync.dma_start(out=vb_sb, in_=vb[:])
            bd_sb = consts.tile([P, 16], F32)
            nc.sync.dma_start(out=bd_sb, in_=bd_dram[:])
            bdT_sb = consts.tile([16, P], F32)
            nc.sync.dma_start(out=bdT_sb, in_=bdT_dram[:])

            ones1 = consts.tile([1, P], F32)
            nc.vector.memset(ones1, 1.0)
            ones32 = consts.tile([P, D], F32)
            nc.vector.memset(ones32, 1.0)
            eps_sb = consts.tile([P, 1], F32)
            nc.vector.memset(eps_sb, EPS)

            # vb broadcast to [P, C] via ones-matmul
            pvb = psum.tile([P, C], F32, tag="acc", bufs=4)
            nc.tensor.matmul(pvb, ones1, vb_sb, start=True, stop=True)
            vb_bc = consts.tile([P, C], F32)
            nc.vector.tensor_copy(out=vb_bc, in_=pvb)

            # ---- projections ----
            q_sb = big.tile([P, 2, NQ], F32)
            for oc in range(2):
                for nb in range(NQ // NB):
                    ps = psum.tile([P, NB], F32, tag="acc", bufs=4)
                    for ic in range(2):
                        nc.tensor.matmul(
                            ps,
                            wq[:, ic, P * oc : P * (oc + 1)],
                            qf_sb[:, ic, NB * nb : NB * (nb + 1)],
                            start=(ic == 0),
                            stop=(ic == 1),
                        )
                    nc.vector.tensor_scalar_add(
                        out=q_sb[:, oc, NB * nb : NB * (nb + 1)],
                        in0=ps,
                        scalar1=qb_sb[:, oc : oc + 1],
                    )

            k_sb = big.tile([P, 2, N], F32)
            for oc in range(2):
                for nb in range(N // NB):
                    ps = psum.tile([P, NB], F32, tag="acc", bufs=4)
                    for ic in range(2):
                        nc.tensor.matmul(
                            ps,
                            wk[:, ic, P * oc : P * (oc + 1)],
                            kf_sb[:, ic, NB * nb : NB * (nb + 1)],
                            start=(ic == 0),
                            stop=(ic == 1),
                        )
                    nc.vector.tensor_scalar_add(
                        out=k_sb[:, oc, NB * nb : NB * (nb + 1)],
                        in0=ps,
                        scalar1=kb_sb[:, oc : oc + 1],
                    )

            # V^T, directly token-major: vt_sb[p, t, o] = V[o, 128*t+p]
            vt_sb = big.tile([P, MCH, C], F32)
            for t in range(MCH):
                ps = psum.tile([P, C], F32, tag="acc", bufs=4)
                for ic in range(2):
                    nc.tensor.matmul(
                        ps,
                        kf_sb[:, ic, P * t : P * (t + 1)],
                        wv[:, ic, :],
                        start=(ic == 0),
                        stop=(ic == 1),
                    )
                nc.vector.tensor_add(out=vt_sb[:, t, :], in0=ps, in1=vb_bc)

            # ---- attention ----
            o_sb = big.tile([P, 2, NQ], F32)
            for g in range(2):
                for nqb in range(NQ // NB):
                    u_ps = psum.tile([P, NB], F32, tag="acc", bufs=4)
                    z_ps = psum.tile([P, NB], F32, tag="acc", bufs=4)
                    for t in range(MCH):
                        s_ps = psum.tile([P, 4 * NB], F32, tag="S", bufs=1)
                        for j in range(4):
                            nc.tensor.matmul(
                                s_ps[:, NB * j : NB * (j + 1)],
                                k_sb[D * j : D * (j + 1), g, P * t : P * (t + 1)],
                                q_sb[D * j : D * (j + 1), g, NB * nqb : NB * (nqb + 1)],
                                start=True,
                                stop=True,
                                tile_position=(D * j, 0),
                            )
                        ex = expp.tile([P, 4 * NB], F32)
                        nc.scalar.activation(
                            out=ex,
                            in_=s_ps,
                            func=mybir.ActivationFunctionType.Exp,
                            scale=SCALE,
                        )
                        for j in range(4):
                            nc.tensor.matmul(
                                u_ps[D * j : D * (j + 1), :],
                                vt_sb[:, t, D * (4 * g + j) : D * (4 * g + j + 1)],
                                ex[:, NB * j : NB * (j + 1)],
                                start=(t == 0),
                                stop=(t == MCH - 1),
                                tile_position=(0, D * j),
                            )
                        for j in range(4):
                            nc.tensor.matmul(
                                z_ps[D * j : D * (j + 1), :],
                                ones32,
                                ex[:, NB * j : NB * (j + 1)],
                                start=(t == 0),
                                stop=(t == MCH - 1),
                                tile_position=(0, D * j),
                            )
                    zi = work.tile([P, NB], F32, tag="zi")
                    nc.vector.reciprocal(out=zi, in_=z_ps)
                    nc.vector.tensor_mul(
                        out=o_sb[:, g, NB * nqb : NB * (nqb + 1)], in0=u_ps, in1=zi
                    )

            # ---- out-projection + residual ----
            t_sb = big.tile([P, 2, NQ], F32)
            for oc in range(2):
                for nb in range(NQ // NB):
                    ps = psum.tile([P, NB], F32, tag="acc", bufs=4)
                    for ic in range(2):
                        nc.tensor.matmul(
                            ps,
                            wo[:, ic, P * oc : P * (oc + 1)],
                            o_sb[:, ic, NB * nb : NB * (nb + 1)],
                            start=(ic == 0),
                            stop=(ic == 1),
                        )
                    dst = t_sb[:, oc, NB * nb : NB * (nb + 1)]
                    nc.vector.tensor_scalar_add(
                        out=dst, in0=ps, scalar1=ob_sb[:, oc : oc + 1]
                    )
                    nc.vector.tensor_add(
                        out=dst, in0=dst, in1=qf_sb[:, oc, NB * nb : NB * (nb + 1)]
                    )

            # ---- GroupNorm stats (per-channel-row, then cross-core) ----
            stats = work.tile([P, 4], F32, tag="stats")
            for oc in range(2):
                bns = work.tile([P, 2, 6], F32, tag="bns")
                for sg in range(2):
                    nc.vector.bn_stats(
                        out=bns[:, sg, :], in_=t_sb[:, oc, 512 * sg : 512 * (sg + 1)]
                    )
                mv = work.tile([P, 2], F32, tag="mv")
                nc.vector.bn_aggr(out=mv, in_=bns)
                nc.vector.tensor_copy(out=stats[:, 2 * oc : 2 * oc + 1], in_=mv[:, 0:1])
                sq = work.tile([P, 1], F32, tag="sq")
                nc.vector.tensor_mul(out=sq, in0=mv[:, 0:1], in1=mv[:, 0:1])
                nc.vector.tensor_add(
                    out=stats[:, 2 * oc + 1 : 2 * oc + 2], in0=mv[:, 1:2], in1=sq
                )

            allst = work.tile([P, 4], F32, tag="allst")
            ccsem = nc.alloc_semaphore("ccsem")
            with tc.tile_critical():
                nc.sync.dma_start(out=cc_in[:], in_=stats).then_inc(ccsem, 16)
                nc.gpsimd.wait_ge(ccsem, 16)
                nc.gpsimd.collective_compute(
                    "AllReduce",
                    mybir.AluOpType.add,
                    replica_groups=_REPLICA_GROUPS,
                    ins=[cc_in[:]],
                    outs=[cc_out[:]],
                ).then_inc(ccsem, 1)
                nc.gpsimd.wait_ge(ccsem, 17)
                nc.gpsimd.dma_start(out=allst, in_=cc_out[:]).then_inc(ccsem, 16)
                nc.gpsimd.wait_ge(ccsem, 33)

            # combine rows -> groups -> back to rows
            gp = psum.tile([16, 4], F32, tag="acc", bufs=4)
            nc.tensor.matmul(gp, bd_sb, allst, start=True, stop=True)
            gs = work.tile([16, 4], F32, tag="gs")
            nc.vector.tensor_copy(out=gs, in_=gp)
            xp = psum.tile([P, 4], F32, tag="acc", bufs=4)
            nc.tensor.matmul(xp, bdT_sb, gs, start=True, stop=True)

            # mean / var / rstd per channel row  (32 row-stats per group)
            meanv = work.tile([P, 2], F32, tag="meanv")
            m2v = work.tile([P, 2], F32, tag="m2v")
            nc.vector.tensor_scalar_mul(
                out=meanv, in0=xp[:, 0:4:2], scalar1=1.0 / 32.0
            )
            nc.vector.tensor_scalar_mul(out=m2v, in0=xp[:, 1:4:2], scalar1=1.0 / 32.0)
            msq = work.tile([P, 2], F32, tag="msq")
            nc.vector.tensor_mul(out=msq, in0=meanv, in1=meanv)
            varv = work.tile([P, 2], F32, tag="varv")
            nc.vector.tensor_tensor(
                out=varv, in0=m2v, in1=msq, op=mybir.AluOpType.subtract
            )
            rstd = work.tile([P, 2], F32, tag="rstd")
            nc.scalar.activation(
                out=rstd,
                in_=varv,
                func=mybir.ActivationFunctionType.Sqrt,
                bias=eps_sb,
                scale=1.0,
            )
            nc.vector.reciprocal(out=rstd, in_=rstd)

            # ---- normalize + write out ----
            y_view = y[:].rearrange("(c p) n -> p c n", p=P)
            for oc in range(2):
                nc.vector.tensor_scalar(
                    out=t_sb[:, oc, :],
                    in0=t_sb[:, oc, :],
                    scalar1=meanv[:, oc : oc + 1],
                    scalar2=rstd[:, oc : oc + 1],
                    op0=mybir.AluOpType.subtract,
                    op1=mybir.AluOpType.mult,
                )
                nc.vector.tensor_scalar(
                    out=t_sb[:, oc, :],
                    in0=t_sb[:, oc, :],
                    scalar1=gnw_sb[:, oc : oc + 1],
                    scalar2=gnb_sb[:, oc : oc + 1],
                    op0=mybir.AluOpType.mult,
                    op1=mybir.AluOpType.add,
                )
                nc.sync.dma_start(out=y_view[:, oc, :], in_=t_sb[:, oc, :])

    _legalize_waits(nc)
    return nc


_NC_CACHE = None


def _get_nc():
    global _NC_CACHE
    if _NC_CACHE is None:
        _NC_CACHE = build_kernel()
    return _NC_CACHE


def _chunk2(v):
    # [256] channel vector -> [128, 2] (row p, chunk oc)
    return np.ascontiguousarray(np.asarray(v, np.float32).reshape(2, P).T)


def make_in_maps(inputs):
    qfeat = np.asarray(inputs["query_feat"], np.float32).reshape(2, C, N)
    kfeat = np.asarray(inputs["key_feat"], np.float32).reshape(2, C, N)
    shared = {
        "qwT": np.ascontiguousarray(np.asarray(inputs["q_w"], np.float32).T),
        "kwT": np.ascontiguousarray(np.asarray(inputs["k_w"], np.float32).T),
        "vwT": np.ascontiguousarray(np.asarray(inputs["v_w"], np.float32).T),
        "owT": np.ascontiguousarray(np.asarray(inputs["out_w"], np.float32).T),
        "qb": _chunk2(inputs["q_b"]),
        "kb": _chunk2(inputs["k_b"]),
        "ob": _chunk2(inputs["out_b"]),
        "vb": np.ascontiguousarray(
            np.asarray(inputs["v_b"], np.float32).reshape(1, C)
        ),
        "gnw": _chunk2(inputs["gn_w"]),
        "gnb": _chunk2(inputs["gn_b"]),
    }
    in_maps = []
    for c in range(8):
        b, s = c // 4, c % 4
        m = dict(shared)
        m["qf"] = np.ascontiguousarray(qfeat[b][:, NQ * s : NQ * (s + 1)])
        m["kf"] = np.ascontiguousarray(kfeat[b])
        in_maps.append(m)
    return in_maps


def run(inputs, **kwargs):
    nc = _get_nc()
    res = run_bass_kernel_spmd(nc, make_in_maps(inputs), core_ids=list(range(8)), **kwargs)
    out = np.empty((2, C, N), np.float32)
    for c in range(8):
        b, s = c // 4, c % 4
        out[b][:, NQ * s : NQ * (s + 1)] = res.results[c]["y"]
    return out.reshape(2, C, 64, 64), res


def kernel(**inputs) -> np.ndarray:
    out, _ = run(inputs)
    return out
